# revision 1
# baseline (speedup 1.0000x reference)
"""Bass/Trainium2 kernel for the decomposed LocallyConnected2d layer.

out[b,o,i,j] = sum_{c,k} x[b, c, i+di, j+dj] * w[o, c, i, j, k] + bias[o,i,j]
with k = di*3 + dj (3x3 kernel, stride 1).

Strategy: shard over output rows i across 8 cores (4 rows each). Each core
owns 1/8 of the per-location weight (the dominant traffic) and a 6-row halo
slice of x. Per output location (i,j) the contraction (c,k)=288 is split into
3 chunks of 96 = (di,c) indexed, chunked over dj; each chunk is one matmul
lhsT=[96,64] rhs=[96,128] accumulating into PSUM [64 o, 128 b]. Even/odd j
use PE column groups 0/1 (tile_position) so two locations' matmuls overlap.
Bias is added during the PSUM->SBUF eviction (fused, alternating DVE/ACT).
All matmul data is fp16 (PE runs fp16 at 4x the fp32 rate; fp32 accumulate
in PSUM), output written fp16 and upcast on host.
"""

import sys

for _p in ("/opt/trn_rl_repo", "/root/.axon_site/_ro/trn_rl_repo"):
    if _p not in sys.path:
        sys.path.append(_p)

import numpy as np

B = 128
C_IN = 32
C_OUT = 64
OH = OW = 32
KH = KW = 3
H = W = 34
N_CORES = 8
RPC = OH // N_CORES          # output rows per core = 4
HALO = RPC + KH - 1          # x rows per core = 6
JH = OW // 2                 # j-pairs per row = 16

_DT_MM = "float16"           # matmul operand dtype
_DT_OUT = "float16"          # device output dtype

_prog_cache = {}


def _build_program():
    import concourse.tile as tile
    from concourse import bacc, mybir

    dt_mm = getattr(mybir.dt, _DT_MM)
    dt_out = getattr(mybir.dt, _DT_OUT)
    f32 = mybir.dt.float32

    nc = bacc.Bacc("TRN2", target_bir_lowering=False, debug=False,
                   num_devices=N_CORES)

    # Per-core DRAM I/O (host pre-sharded / pre-transposed):
    #   x_in  [c=32, h=6, w=34, b=128]  halo slice, b innermost
    #   w_in  [i=4, p=96 (di*32+c), dj=3, j=32, o=64]
    #   b_in  [p2=128 (par*64+o), i=4, jh=16] fp32
    #   out   [p2=128 (par*64+o), i=4, jh=16, b=128] ; j = 2*jh + par
    x_in = nc.dram_tensor("x", [C_IN, HALO, W, B], dt_mm,
                          kind="ExternalInput").ap()
    w_in = nc.dram_tensor("w", [RPC, 96, KW, OW, C_OUT], dt_mm,
                          kind="ExternalInput").ap()
    b_in = nc.dram_tensor("bias", [128, RPC, JH], f32,
                          kind="ExternalInput").ap()
    out = nc.dram_tensor("out", [128, RPC, JH, B], dt_out,
                         kind="ExternalOutput").ap()

    Ident = mybir.ActivationFunctionType.Identity

    with tile.TileContext(nc) as tc:
        with (
            tc.tile_pool(name="xpool", bufs=1) as xpool,
            tc.tile_pool(name="wpool", bufs=2) as wpool,
            tc.tile_pool(name="bpool", bufs=1) as bpool,
            tc.tile_pool(name="opool", bufs=2) as opool,
            tc.tile_pool(name="pspool", bufs=6, space="PSUM") as pspool,
        ):
            bias_sb = bpool.tile([128, RPC, JH], f32)
            nc.gpsimd.dma_start(bias_sb[:], b_in[:])

            # x slabs: one tile per output row Delta; partition p = di*32+c
            # holds image row (Delta+di) so every matmul chunk reads a single
            # compile-time free offset.
            xslabs = []
            for d in range(RPC):
                xs = xpool.tile([96, W, B], dt_mm, tag=f"xs{d}")
                for di in range(KH):
                    nc.gpsimd.dma_start(xs[32 * di:32 * di + 32, :, :],
                                        x_in[:, d + di, :, :])
                xslabs.append(xs)

            for i in range(RPC):
                w_t = wpool.tile([96, KW, OW, C_OUT], dt_mm)
                nc.sync.dma_start(w_t[:], w_in[i])

                out_row = opool.tile([128, JH, B], dt_out)
                xs = xslabs[i]
                for jh in range(JH):
                    ps = pspool.tile([128, B], f32)
                    for par in range(2):
                        j = 2 * jh + par
                        pslice = ps[64 * par:64 * par + 64, :]
                        for dj in range(KW):
                            nc.tensor.matmul(
                                pslice,
                                w_t[:, dj, j, :],
                                xs[:, j + dj, :],
                                start=(dj == 0),
                                stop=(dj == KW - 1),
                                tile_position=(0, 64 * par),
                            )
                    dst = out_row[:, jh, :]
                    bs = bias_sb[:, i, jh:jh + 1]
                    if jh % 2 == 0:
                        nc.vector.tensor_scalar_add(dst, ps[:], bs)
                    else:
                        nc.scalar.activation(dst, ps[:], Ident, bias=bs)
                nc.sync.dma_start(out[:, i, :, :], out_row[:])

    nc.compile()
    return nc


def _host_prep(x, weight, bias):
    """Full fp32 inputs -> list of per-core input dicts."""
    np_mm = np.dtype(_DT_MM)
    # x: (B, C, H, W) -> (C, H, W, B)
    x_t = np.ascontiguousarray(x.transpose(1, 2, 3, 0)).astype(np_mm)
    # w: (O, C, I, J, K) -> [i, di, c, dj, j, o] -> (I, 96, KW, J, O)
    w_r = weight.reshape(C_OUT, C_IN, OH, OW, KH, KW)
    w_t = w_r.transpose(2, 4, 1, 5, 3, 0).reshape(OH, 96, KW, OW, C_OUT)
    w_t = np.ascontiguousarray(w_t).astype(np_mm)
    # bias: (O, I, J) -> [par*64+o, i, jh]
    b_t = bias.reshape(C_OUT, OH, JH, 2).transpose(3, 0, 1, 2)
    b_t = np.ascontiguousarray(b_t.reshape(128, OH, JH), dtype=np.float32)

    in_maps = []
    for m in range(N_CORES):
        r0 = m * RPC
        in_maps.append({
            "x": np.ascontiguousarray(x_t[:, r0:r0 + HALO]),
            "w": np.ascontiguousarray(w_t[r0:r0 + RPC]),
            "bias": np.ascontiguousarray(b_t[:, r0:r0 + RPC]),
        })
    return in_maps


def _gather(results):
    out_full = np.empty((B, C_OUT, OH, OW), np.float32)
    for m in range(N_CORES):
        r = results[m]["out"].astype(np.float32)          # (128, 4, 16, 128)
        r = r.reshape(2, C_OUT, RPC, JH, B)               # par,o,i,jh,b
        r = r.transpose(4, 1, 2, 3, 0)                    # b,o,i,jh,par
        out_full[:, :, m * RPC:(m + 1) * RPC, :] = r.reshape(B, C_OUT, RPC, OW)
    return out_full


def kernel(x, weight, bias, _trace=False):
    from concourse.bass_utils import run_bass_kernel_spmd

    if "nc" not in _prog_cache:
        _prog_cache["nc"] = _build_program()
    nc = _prog_cache["nc"]

    in_maps = _host_prep(np.asarray(x), np.asarray(weight), np.asarray(bias))
    res = run_bass_kernel_spmd(nc, in_maps, core_ids=list(range(N_CORES)),
                               trace=_trace)
    out = _gather(res.results)
    if _trace:
        _prog_cache["last_result"] = res
    return out
